# revision 11
# baseline (speedup 1.0000x reference)
"""int4 weight-only quantized GEMV on 8 TRN2 NeuronCores.

out[1, n] = sum_k A[1, k] * W[n, k],   W = dequant(B packed nibbles, scales/zeros)
A: [1, 8192] fp16, B: [16384, 4096] int32 (one byte per elem, 2 nibbles),
scalesAndZeros: [16384, 256, 2] fp16 (group=32 along K).

Sharding: N=16384 rows split across 8 cores (2048 rows each); A replicated.

Scheme (per core):
  Host packs B's bytes as uint16 words (4 nibbles of 4 consecutive k per word),
  TRANSPOSED to [2048 kq, 2048 n] with kq rows permuted so each 128-row tile
  covers 32 groups x 4 words.  The device turns each word-tile into 4 fp16
  "planes" with ONE cheap DVE op each:
    p_j = (x & (0xF<<4j)) | 0x4000   (j=0..2)  -> fp16 value 2 + n_j/2^(9-4j)
    p_3 = x * 2^-12                           -> n_3 + lower-nibble fraction
  The planes feed fp16 matmuls (stationary = 32-col block-diagonal tables of
  corrected A coefficients c_j; correction cancels the affine/fraction terms),
  accumulating P_raw[g, n] in PSUM.  Epilogue: subtract per-group constant K_g
  (DVE, per-partition scalar), multiply by scales^T (DVE), reduce over g plus
  the zeros/Asum channels via 1-column matmuls, copy out.
"""

import numpy as np

import concourse.bass as bass
import concourse.bacc as bacc
import concourse.mybir as mybir
from concourse import tile
from concourse.bass_utils import run_bass_kernel_spmd

FP16 = mybir.dt.float16
FP32 = mybir.dt.float32
U16 = mybir.dt.uint16
Alu = mybir.AluOpType

M, K, N = 1, 8192, 16384
KH = K // 2          # 4096 packed bytes per row
GROUP = 32
NG = K // GROUP      # 256 groups
NCORES = 8
NS = N // NCORES     # 2048 rows per core
KQ = K // 4          # 2048 uint16 words per row
P = 128
NT = KQ // P         # 16 word-tiles per core
NH = 2               # n processed in halves (PSUM capacity)
HW = NS // NH        # 1024 columns per half


def build_program():
    nc = bacc.Bacc()
    w_d = nc.declare_dram_parameter("W", [KQ, NS], U16, isOutput=False)
    ct_d = nc.declare_dram_parameter("CT", [P, NT * 4 * 32], FP16, isOutput=False)
    kg_d = nc.declare_dram_parameter("KG", [2, P, 1], FP32, isOutput=False)
    sa_d = nc.declare_dram_parameter("SAV", [2, P, 1], FP16, isOutput=False)
    u_d = nc.declare_dram_parameter("UV", [2, P, 1], FP16, isOutput=False)
    st_d = nc.declare_dram_parameter("ST", [2, P, NS], FP16, isOutput=False)
    zt_d = nc.declare_dram_parameter("ZT", [2, P, NS], FP16, isOutput=False)
    out_d = nc.declare_dram_parameter("OUT", [NS], FP16, isOutput=True)

    with tile.TileContext(nc) as tc:
        with (
            tc.tile_pool(name="const", bufs=1) as cpool,
            tc.tile_pool(name="bin", bufs=3) as bpool,
            tc.tile_pool(name="work", bufs=2) as wpool,
            tc.tile_pool(name="small", bufs=2) as spool,
            tc.tile_pool(name="psum", bufs=1, space="PSUM") as ppool,
        ):
            ct = cpool.tile([P, NT * 4 * 32], FP16)
            nc.sync.dma_start(out=ct[:, :], in_=ct_d[:, :])
            kg = []
            sa = []
            uv = []
            st = []
            zt = []
            for r in range(2):
                kg.append(cpool.tile([P, 1], FP32, name=f"kg{r}"))
                nc.sync.dma_start(out=kg[r][:, :], in_=kg_d[r])
                sa.append(cpool.tile([P, 1], FP16, name=f"sa{r}"))
                nc.sync.dma_start(out=sa[r][:, :], in_=sa_d[r])
                uv.append(cpool.tile([P, 1], FP16, name=f"uv{r}"))
                nc.sync.dma_start(out=uv[r][:, :], in_=u_d[r])
                st.append(cpool.tile([P, NS], FP16, name=f"st{r}"))
                nc.sync.dma_start(out=st[r][:, :], in_=st_d[r])
                zt.append(cpool.tile([P, NS], FP16, name=f"zt{r}"))
                nc.sync.dma_start(out=zt[r][:, :], in_=zt_d[r])
            ones = cpool.tile([P, 1], FP16)
            nc.vector.memset(ones[:, :], 1.0)

            for h in range(NH):
                c0 = h * HW
                pa = ppool.tile([P, HW], FP32, tag="pa")
                pb = ppool.tile([P, HW], FP32, tag="pb")
                po = ppool.tile([1, HW], FP32, tag="po")
                preg = [pa, pb]

                for T in range(NT):
                    bt = bpool.tile([P, HW], U16, tag="bt")
                    nc.gpsimd.dma_start(
                        out=bt[:, :], in_=w_d[T * P : (T + 1) * P, c0 : c0 + HW]
                    )
                    pl = wpool.tile([P, 4 * HW], U16, tag="pl")
                    for j, msk in enumerate((0x000F, 0x00F0)):
                        nc.vector.tensor_scalar(
                            out=pl[:, j * HW : (j + 1) * HW],
                            in0=bt[:, :],
                            scalar1=msk, scalar2=0x4000,
                            op0=Alu.bitwise_and, op1=Alu.bitwise_or,
                        )
                    tt = wpool.tile([P, HW], U16, tag="tt")
                    nc.vector.tensor_scalar(
                        out=tt[:, :], in0=bt[:, :],
                        scalar1=0xFF00, scalar2=8,
                        op0=Alu.bitwise_and, op1=Alu.logical_shift_right,
                    )
                    nc.vector.tensor_scalar(
                        out=pl[:, 2 * HW : 3 * HW], in0=tt[:, :],
                        scalar1=0x000F, scalar2=0x4000,
                        op0=Alu.bitwise_and, op1=Alu.bitwise_or,
                    )
                    nc.vector.tensor_scalar(
                        out=pl[:, 3 * HW : 4 * HW], in0=tt[:, :],
                        scalar1=0x00F0, scalar2=0x4000,
                        op0=Alu.bitwise_and, op1=Alu.bitwise_or,
                    )

                    f, cT = T // 2, T % 2
                    R, base = f // 4, 32 * (f % 4)
                    pt = preg[R]
                    for j in range(4):
                        for q in range(HW // 512):
                            nc.tensor.matmul(
                                out=pt[base : base + 32, q * 512 : (q + 1) * 512],
                                lhsT=ct[:, (T * 4 + j) * 32 : (T * 4 + j + 1) * 32],
                                rhs=pl[:, j * HW + q * 512 : j * HW + (q + 1) * 512].bitcast(FP16),
                                start=(cT == 0 and j == 0),
                                stop=(cT == 1 and j == 3),
                                tile_position=(0, base),
                            )

                # epilogue for this half
                prods = []
                for r in range(2):
                    pc = spool.tile([P, HW], FP16, tag=f"pc{r}")
                    nc.vector.tensor_scalar(
                        out=pc[:, :], in0=preg[r][:, :],
                        scalar1=kg[r][:, :], scalar2=None, op0=Alu.subtract,
                    )
                    prod = spool.tile([P, HW], FP16, tag=f"prod{r}")
                    nc.vector.tensor_tensor(
                        out=prod[:, :], in0=pc[:, :],
                        in1=st[r][:, c0 : c0 + HW], op=Alu.mult,
                    )
                    prods.append(prod)

                for q in range(HW // 512):
                    cq = slice(q * 512, (q + 1) * 512)
                    cqg = slice(c0 + q * 512, c0 + (q + 1) * 512)
                    chan = (
                        [(ones, prods[r][:, cq]) for r in range(2)]
                        + [(uv[r], st[r][:, cqg]) for r in range(2)]
                        + [(sa[r], zt[r][:, cqg]) for r in range(2)]
                    )
                    for i, (lhsT, rhs) in enumerate(chan):
                        nc.tensor.matmul(
                            out=po[0:1, cq], lhsT=lhsT[:, :], rhs=rhs,
                            start=(i == 0), stop=(i == len(chan) - 1),
                            tile_position=(0, 0),
                        )

                outt = spool.tile([1, HW], FP16, tag="outt")
                nc.vector.tensor_copy(out=outt[:, :], in_=po[0:1, :])
                nc.gpsimd.dma_start(out=out_d[c0 : c0 + HW], in_=outt[0:1, :])
    nc.finalize()
    return nc


_NC_CACHE = {}


def _get_program():
    if "nc" not in _NC_CACHE:
        _NC_CACHE["nc"] = build_program()
    return _NC_CACHE["nc"]


def _perm_kq():
    T = np.arange(NT)[:, None]
    i = np.arange(P)[None, :]
    G = 32 * (T // 2) + i // 4
    return (8 * G + 4 * (T % 2) + (i % 4)).reshape(-1)


_PREP_CACHE = {}


def _prep_shared(A):
    """A-dependent, core-independent prep: c tables, K_g, SA, U."""
    a = np.asarray(A).reshape(K).astype(np.float64)
    idx = _perm_kq()
    a4 = a.reshape(KQ, 4)
    c = np.empty((KQ, 4), np.float64)
    c[:, 0] = 512.0 * a4[:, 0]
    c[:, 1] = 32.0 * a4[:, 1]
    c[:, 2] = 512.0 * a4[:, 2]
    c[:, 3] = 32.0 * a4[:, 3]
    c16 = c.astype(np.float16)
    c64 = c16.astype(np.float64)

    # CT[i, (T*4+j)*32 + col] = c16[kq(T,i), j] if col == i//4 else 0
    ct = np.zeros((P, NT * 4 * 32), np.float16)
    kq = idx.reshape(NT, P)
    ii = np.arange(P)
    for T in range(NT):
        for j in range(4):
            ct[ii, (T * 4 + j) * 32 + ii // 4] = c16[kq[T], j]

    csum = 2.0 * c64.sum(-1)
    Kg = csum.reshape(NG, 8).sum(-1).astype(np.float32).reshape(2, P, 1)

    SA = a.reshape(NG, GROUP).sum(-1)
    sav = SA.astype(np.float16).reshape(2, P, 1)
    uvv = (-8.0 * SA).astype(np.float16).reshape(2, P, 1)
    return ct, Kg, sav, uvv, idx


def build_in_maps(A, B, SZ):
    A = np.asarray(A)
    B = np.asarray(B)
    SZ = np.asarray(SZ)

    ct, Kg, sav, uvv, idx = _prep_shared(A)

    b8 = B.astype(np.uint8)                       # [N, KH]
    w16 = b8.view(np.uint16)                      # [N, KQ]

    in_maps = []
    for c in range(NCORES):
        r0 = c * NS
        wt = np.ascontiguousarray(w16[r0 : r0 + NS].T[idx])     # [KQ, NS]
        sT = np.ascontiguousarray(
            SZ[r0 : r0 + NS, :, 0].T.reshape(2, P, NS)
        )
        zT = np.ascontiguousarray(
            SZ[r0 : r0 + NS, :, 1].T.reshape(2, P, NS)
        )
        in_maps.append(
            {
                "W": wt,
                "CT": ct,
                "KG": Kg,
                "SAV": sav,
                "UV": uvv,
                "ST": sT,
                "ZT": zT,
            }
        )
    return in_maps


def kernel(A, B, scalesAndZeros):
    nc = _get_program()
    in_maps = build_in_maps(A, B, scalesAndZeros)
    res = run_bass_kernel_spmd(nc, in_maps, core_ids=list(range(NCORES)))
    out = np.concatenate([res.results[c]["OUT"] for c in range(NCORES)])
    return out.reshape(1, N).astype(np.float16)


if __name__ == "__main__":
    rng = np.random.default_rng(0)
    A = rng.standard_normal((M, K)).astype(np.float16)
    B = rng.integers(0, 256, (N, KH)).astype(np.int32)
    SZ = rng.standard_normal((N, NG, 2)).astype(np.float16)
    out = kernel(A, B, SZ)
    print(out.shape, out.dtype, out[0, :8])
